# revision 40
# baseline (speedup 1.0000x reference)
"""Trainium2 Bass kernel for nn_Attention_84585085927925 — folded Gram chain.

Reference (per batch element b, fp32):
    qkv = x @ w_qkv.T ; q,k,v heads of 64 ; attn = sqrt(64) * q @ k.T (NO
    softmax) ; out = attn @ v ; out = out @ w_fc.T + b_fc

No softmax -> the map is linear and collapses into a per-batch weight:
    G_h  = wk_h (x^T x) wv_h^T                (64 x 64 per head)
    W    = sum_h (s*wq_h)^T G_h wfc_h^T       (768 x 768)
    out  = x @ W + b_fc
Per-core pipeline (one batch element per core, 8 cores):
    C    = x^T x        symmetric: 21 upper blocks (bf16) + 15 PE transposes
    M1T  = C @ wv^T     (f32r, = (wv C)^T)
    G'_t = diag 64-blocks of M1T_pair^T wk_pair  (bf16, narrow)
    GF   = blockdiag(G') @ wfc^T                 (f32r)
    W    = (s*wq)^T @ GF                         (f32r)
    outT = W^T-tiles @ x^T + bias                (f32r)
Measured rates: f32r matmul ~0.44 ns/col (>=256 wide), bf16 ~0.505 ns/col
(any width).  f32r for wide phases, bf16 for C (cheap x streaming, narrow
C blocks) and G'.  HBM tensors are host-prearranged to [128, ...] so every
DMA is 128 contiguous descriptors; DMA triggers cost ~700ns serialized on
their queue engine, so output triggers alternate sync/gpsimd.
"""

import numpy as np
import ml_dtypes

import concourse.bass as bass  # noqa: F401  (registers engine namespaces)
import concourse.mybir as mybir
import concourse.tile as tile
from concourse import bacc, bass_utils

F32 = mybir.dt.float32
F32R = mybir.dt.float32r
BF16 = mybir.dt.bfloat16

B, N, D, H = 8, 1024, 768, 12
HD = D // H            # 64
SCALE = float(np.sqrt(HD))
DT = D // 128          # 6 chunks of the feature axis
NT = N // 128          # 8 token tiles
NPAIR = H // 2         # 6 head pairs
ECH = 384              # column chunk (fits PSUM bank with headroom)


def _build_program():
    nc = bacc.Bacc(
        trn_type="TRN2", target_bir_lowering=False, debug=False, num_devices=B
    )
    xn_d = nc.dram_tensor("xn", [128, NT, D], BF16, kind="ExternalInput").ap()
    xt_d = nc.dram_tensor("xt", [128, DT, N], F32, kind="ExternalInput").ap()
    wvt_d = nc.dram_tensor("wvt", [128, DT, D], F32, kind="ExternalInput").ap()
    wkt_d = nc.dram_tensor("wkt", [128, DT, D], BF16,
                           kind="ExternalInput").ap()
    wq_d = nc.dram_tensor("wq", [128, DT, D], F32, kind="ExternalInput").ap()
    wfct_d = nc.dram_tensor("wfct", [128, DT, D], F32,
                            kind="ExternalInput").ap()
    idm_d = nc.dram_tensor("idm", [128, 128], BF16, kind="ExternalInput").ap()
    bias_d = nc.dram_tensor("bias", [128, DT], F32, kind="ExternalInput").ap()
    out_d = nc.dram_tensor("outT", [128, 12, 512], BF16,
                           kind="ExternalOutput").ap()

    with tile.TileContext(nc) as tc:
        with tc.tile_pool(name="big", bufs=1) as big, \
             tc.tile_pool(name="tmps", bufs=16) as tmps, \
             tc.tile_pool(name="outsp", bufs=4) as outsp, \
             tc.tile_pool(name="ps", bufs=7, space="PSUM") as ps:

            xn_sb = big.tile([128, NT, D], BF16, name="xn_sb")
            xt_sb = big.tile([128, DT, N], F32R, name="xt_sb")
            wvt_sb = big.tile([128, DT, D], F32R, name="wvt_sb")
            wkt_sb = big.tile([128, DT, D], BF16, name="wkt_sb")
            wq_sb = big.tile([128, DT, D], F32R, name="wq_sb")
            wfct_sb = big.tile([128, DT, D], F32R, name="wfct_sb")
            c_sb = big.tile([128, DT, D], F32R, name="c_sb")
            m1t_sb = big.tile([128, DT, D], BF16, name="m1t_sb")
            g2_sb = big.tile([128, NPAIR, 128], F32R, name="g2_sb")
            gf_sb = big.tile([128, DT, D], F32R, name="gf_sb")
            w_sb = big.tile([128, DT, D], F32R, name="w_sb")
            id_sb = big.tile([128, 128], BF16, name="id_sb")
            bias_sb = big.tile([128, DT], F32, name="bias_sb")
            warm_sb = big.tile([128, 512], BF16, name="warm_sb")

            xtr = xt_d.bitcast(F32R)
            wvtr = wvt_d.bitcast(F32R)
            wqr = wq_d.bitcast(F32R)
            wfctr = wfct_d.bitcast(F32R)

            # ---- DMA triggers: alternate sync/scalar so the ~700ns
            # per-trigger costs overlap; x first (C needs it), small
            # leading chunks so streaming keeps pace with pass 1 ----
            nc.sync.dma_start(xn_sb[:, 0:1, :], xn_d[:, 0:1, :])
            nc.scalar.dma_start(xn_sb[:, 1:2, :], xn_d[:, 1:2, :])
            nc.sync.dma_start(xn_sb[:, 2:3, :], xn_d[:, 2:3, :])
            nc.scalar.dma_start(xn_sb[:, 3:5, :], xn_d[:, 3:5, :])
            nc.sync.dma_start(xn_sb[:, 5:8, :], xn_d[:, 5:8, :])
            nc.scalar.dma_start(id_sb[:], idm_d[:])
            nc.sync.dma_start(wvt_sb[:], wvtr[:])
            nc.scalar.dma_start(wkt_sb[:], wkt_d[:])
            nc.sync.dma_start(wfct_sb[:], wfctr[:])
            nc.scalar.dma_start(wq_sb[:], wqr[:])
            nc.sync.dma_start(xt_sb[:], xtr[:])
            nc.scalar.dma_start(bias_sb[:], bias_d[:])

            # ---- PE warmup (p-state ramp) while x streams in ----
            nc.vector.memset(warm_sb[:], 0.0)
            wpt = ps.tile([128, 512], F32, tag="b", name="wpt")
            for i in range(7):
                nc.tensor.matmul(wpt[:], warm_sb[:, 0:128], warm_sb[:],
                                 start=(i == 0), stop=(i == 6))

            # ---- C = x^T x, upper triangle in two passes (bf16).
            # Pass 1 (rows 0-2, cols 384:768) consumes x at roughly its
            # DMA arrival rate; pass 2 covers the rest of the upper
            # triangle starting at the diagonal.  Lower blocks come from
            # PE transposes emitted inside the M1T descent so the chains
            # that need them (small d, late) never wait.
            tmt = {}   # (d1, d2) -> bf16 staging tile for transpose

            def c_pass(specs):
                # specs: list of (d1, cs, ce) accumulator strips
                accs = {}
                for d1, cs, ce in specs:
                    acc = ps.tile([128, ce - cs], F32, tag="b",
                                  name=f"cp{d1}_{cs}")
                    accs[d1] = (acc, cs)
                for nt in range(NT):
                    for d1, cs, ce in specs:
                        acc = accs[d1][0]
                        nc.tensor.matmul(
                            acc[:],
                            xn_sb[:, nt, d1 * 128:(d1 + 1) * 128],
                            xn_sb[:, nt, cs:ce],
                            start=(nt == 0), stop=(nt == NT - 1),
                        )
                return accs

            def c_pass_copies(accs, ce_of, copy_order):
                # copy_order: list of ('i', d1) full-strip or ('t', d1, d2)
                # transpose-staging entries, alternating vector/scalar
                for i, ent in enumerate(copy_order):
                    eng_v = (i % 2 == 0)
                    if ent[0] == 'i':
                        d1 = ent[1]
                        acc, cs = accs[d1]
                        dst = c_sb[:, d1, cs:ce_of[d1]]
                        if eng_v:
                            nc.vector.tensor_copy(dst, acc[:])
                        else:
                            nc.scalar.copy(dst, acc[:])
                    else:
                        d1, d2 = ent[1], ent[2]
                        acc, cs = accs[d1]
                        tm = tmps.tile([128, 128], BF16, tag="tm",
                                       name=f"tm{d1}{d2}")
                        src = acc[:, d2 * 128 - cs:(d2 + 1) * 128 - cs]
                        if eng_v:
                            nc.vector.tensor_copy(tm[:], src)
                        else:
                            nc.scalar.copy(tm[:], src)
                        tmt[(d1, d2)] = tm

            def c_transpose(pairs):
                for d1, d2 in pairs:
                    pt = ps.tile([128, 128], BF16, tag="b", name=f"tp{d1}{d2}")
                    nc.tensor.transpose(pt[:], tmt.pop((d1, d2)), id_sb[:])
                    dst = c_sb[:, d2, d1 * 128:(d1 + 1) * 128]
                    if d2 % 2 == 0:
                        nc.scalar.copy(dst, pt[:])
                    else:
                        nc.vector.tensor_copy(dst, pt[:])

            accs_1 = c_pass([(0, ECH, D), (1, ECH, D), (2, ECH, D)])
            c_pass_copies(accs_1, {0: D, 1: D, 2: D},
                          [('i', 0), ('t', 0, 3), ('t', 0, 4), ('t', 0, 5),
                           ('i', 1), ('t', 1, 3), ('t', 1, 4), ('t', 1, 5),
                           ('i', 2), ('t', 2, 3), ('t', 2, 4), ('t', 2, 5)])
            accs_2 = c_pass([(5, 640, D), (4, 512, D), (3, ECH, D),
                             (0, 0, ECH), (1, 128, ECH), (2, 256, ECH)])
            c_pass_copies(accs_2, {5: D, 4: D, 3: D, 0: ECH, 1: ECH, 2: ECH},
                          [('i', 5), ('i', 4), ('t', 4, 5),
                           ('i', 3), ('t', 3, 4), ('t', 3, 5),
                           ('i', 0), ('t', 0, 1), ('t', 0, 2),
                           ('i', 1), ('t', 1, 2), ('i', 2)])

            # pass-1-sourced transposes (staging ready since mid-pass-2);
            # the six pass-2-sourced ones run after M1T d=5, whose 2.1us
            # shadow lets their staging copies land
            c_transpose([(2, 3), (2, 4), (2, 5), (1, 3), (1, 4), (1, 5),
                         (0, 3), (0, 4), (0, 5)])

            # ---- M1T = C @ wv^T (f32r); d descending; per-d k order
            # puts freshly transposed lower blocks at the chain's end ----
            m1_korder = {
                5: [5, 4, 3, 2, 1, 0],
                4: [4, 3, 2, 1, 0, 5],
                3: [3, 2, 1, 0, 4, 5],
            }
            for d in range(DT - 1, -1, -1):
                for jc in range(2):
                    pt = ps.tile([128, ECH], F32, tag="b", name="pt_m1")
                    ks = m1_korder.get(d, list(range(DT - 1, -1, -1)))
                    for i, k in enumerate(ks):
                        nc.tensor.matmul(
                            pt[:],
                            c_sb[:, k, d * 128:(d + 1) * 128],
                            wvt_sb[:, k, jc * ECH:(jc + 1) * ECH],
                            start=(i == 0), stop=(i == DT - 1),
                        )
                    dst = m1t_sb[:, d, jc * ECH:(jc + 1) * ECH]
                    if jc == 0:
                        nc.vector.tensor_copy(dst, pt[:])
                    else:
                        nc.scalar.copy(dst, pt[:])
                if d == DT - 1:
                    c_transpose([(4, 5), (3, 4), (3, 5),
                                 (0, 1), (0, 2), (1, 2)])

            # ---- G' per head pair: diag 64-blocks of M1T_p^T @ wkT_p ----
            for t in range(NPAIR):
                gp = ps.tile([128, 128], F32, tag="b", name="gp")
                for k in range(DT - 1, -1, -1):
                    nc.tensor.matmul(
                        gp[:],
                        m1t_sb[:, k, t * 128:(t + 1) * 128],
                        wkt_sb[:, k, t * 128:(t + 1) * 128],
                        start=(k == DT - 1), stop=(k == 0),
                    )
                nc.vector.tensor_scalar_mul(g2_sb[:, t, :], gp[:], 0.0)
                nc.vector.tensor_copy(g2_sb[0:64, t, 0:64], gp[0:64, 0:64])
                nc.vector.tensor_copy(g2_sb[64:128, t, 64:128],
                                      gp[64:128, 64:128])

            # ---- GF = blockdiag(G') @ wfc^T (f32r) ----
            for jc in range(2):
                for t in range(NPAIR):
                    pt = ps.tile([128, ECH], F32, tag="b", name="pt_gf")
                    nc.tensor.matmul(
                        pt[:],
                        g2_sb[:, t, :],
                        wfct_sb[:, t, jc * ECH:(jc + 1) * ECH],
                        start=True, stop=True,
                    )
                    dst = gf_sb[:, t, jc * ECH:(jc + 1) * ECH]
                    if t % 2 == 0:
                        nc.vector.tensor_copy(dst, pt[:])
                    else:
                        nc.scalar.copy(dst, pt[:])

            # ---- W = (s*wq)^T @ GF (f32r) ----
            for jc in range(2):
                for d in range(DT):
                    pt = ps.tile([128, ECH], F32, tag="b", name="pt_w")
                    for k in range(DT):
                        nc.tensor.matmul(
                            pt[:],
                            wq_sb[:, k, d * 128:(d + 1) * 128],
                            gf_sb[:, k, jc * ECH:(jc + 1) * ECH],
                            start=(k == 0), stop=(k == DT - 1),
                        )
                    dst = w_sb[:, d, jc * ECH:(jc + 1) * ECH]
                    if d % 2 == 0:
                        nc.vector.tensor_copy(dst, pt[:])
                    else:
                        nc.scalar.copy(dst, pt[:])

            # ---- outT[j, n] = sum_d W[d, j] x^T[d, n] + bias ----
            for jt in range(DT):
                for ic in range(2):
                    last = (jt == DT - 1 and ic == 1)
                    if not last:
                        pt = ps.tile([128, 512], F32, tag="b", name="pt_o")
                        for k in range(DT):
                            nc.tensor.matmul(
                                pt[:],
                                w_sb[:, k, jt * 128:(jt + 1) * 128],
                                xt_sb[:, k, ic * 512:(ic + 1) * 512],
                                start=(k == 0), stop=(k == DT - 1),
                            )
                        ot = outsp.tile([128, 512], BF16, tag="ot",
                                        name="ot")
                        # add + trigger both on scalar: no cross-engine hop
                        nc.scalar.add(ot[:], pt[:], bias_sb[:, jt:jt + 1])
                        nc.scalar.dma_start(out_d[:, jt * 2 + ic, :], ot[:])
                    else:
                        # final piece in two halves so the first DMA
                        # overlaps the second half's matmuls
                        for h in range(2):
                            off = ic * 512 + h * 256
                            pt = ps.tile([128, 256], F32, tag="b",
                                         name="pt_ol")
                            for k in range(DT):
                                nc.tensor.matmul(
                                    pt[:],
                                    w_sb[:, k, jt * 128:(jt + 1) * 128],
                                    xt_sb[:, k, off:off + 256],
                                    start=(k == 0), stop=(k == DT - 1),
                                )
                            ot = outsp.tile([128, 256], BF16, tag="otl",
                                            bufs=2, name="otl")
                            if h == 0:
                                nc.scalar.add(ot[:], pt[:],
                                              bias_sb[:, jt:jt + 1])
                                nc.scalar.dma_start(
                                    out_d[:, jt * 2 + ic, 0:256], ot[:])
                            else:
                                nc.vector.tensor_scalar_add(
                                    ot[:], pt[:], bias_sb[:, jt:jt + 1])
                                nc.sync.dma_start(
                                    out_d[:, jt * 2 + ic, 256:512], ot[:])

    nc.compile()
    return nc


_NC_CACHE = None
LAST_EXEC_NS = None
LAST_RES = None


def _arr128(a):
    """[D0*128, M] row-major -> [128, D0, M] partition-major, contiguous."""
    d0 = a.shape[0] // 128
    return np.ascontiguousarray(
        a.reshape(d0, 128, a.shape[1]).transpose(1, 0, 2))


def kernel(x, w_qkv, w_fc, b_fc, _trace=False):
    global _NC_CACHE, LAST_EXEC_NS, LAST_RES
    x = np.asarray(x, dtype=np.float32)
    w_qkv = np.asarray(w_qkv, dtype=np.float32)
    w_fc = np.asarray(w_fc, dtype=np.float32)
    b_fc = np.asarray(b_fc, dtype=np.float32)

    if _NC_CACHE is None:
        _NC_CACHE = _build_program()
    nc = _NC_CACHE

    bf = ml_dtypes.bfloat16
    wq = _arr128(SCALE * w_qkv[:D])                       # [128, 6, 768] f32
    wkt = _arr128(np.ascontiguousarray(w_qkv[D:2 * D].T).astype(bf))
    wvt = _arr128(np.ascontiguousarray(w_qkv[2 * D:].T))
    wfct = _arr128(np.ascontiguousarray(w_fc.T))
    bias = np.ascontiguousarray(b_fc.reshape(DT, 128).T)  # [128, 6] f32
    idm = np.eye(128, dtype=bf)

    in_maps = []
    for b in range(B):
        in_maps.append({
            "xn": _arr128(x[b].astype(bf)),               # [128, 8, 768]
            "xt": _arr128(np.ascontiguousarray(x[b].T)),  # [128, 6, 1024] f32
            "wvt": wvt, "wkt": wkt, "wq": wq, "wfct": wfct,
            "idm": idm, "bias": bias,
        })

    res = bass_utils.run_bass_kernel_spmd(
        nc, in_maps, core_ids=list(range(B)), trace=_trace
    )
    LAST_EXEC_NS = res.exec_time_ns
    LAST_RES = res
    outs = []
    for b in range(B):
        a = res.results[b]["outT"]                        # [128, 12, 512] bf16
        a = a.reshape(128, DT, 2, 512).transpose(1, 0, 2, 3).reshape(D, N)
        outs.append(a.T.astype(np.float32))
    return np.ascontiguousarray(np.stack(outs))


# revision 46
# speedup vs baseline: 1.0444x; 1.0444x over previous
"""Trainium2 Bass kernel for nn_Attention_84585085927925 — folded Gram chain.

Reference (per batch element b, fp32):
    qkv = x @ w_qkv.T ; q,k,v heads of 64 ; attn = sqrt(64) * q @ k.T (NO
    softmax) ; out = attn @ v ; out = out @ w_fc.T + b_fc

No softmax -> the map is linear and collapses into a per-batch weight:
    G_h  = wk_h (x^T x) wv_h^T                (64 x 64 per head)
    W    = sum_h (s*wq_h)^T G_h wfc_h^T       (768 x 768)
    out  = x @ W + b_fc
Per-core pipeline (one batch element per core, 8 cores):
    C    = x^T x        symmetric: 21 upper blocks (bf16) + 15 PE transposes
    M1T  = C @ wv^T     (f32r, = (wv C)^T)
    G'_t = diag 64-blocks of M1T_pair^T wk_pair  (bf16, narrow)
    GF   = blockdiag(G') @ wfc^T                 (f32r)
    W    = (s*wq)^T @ GF                         (f32r)
    outT = W^T-tiles @ x^T + bias                (f32r)
Measured rates: f32r matmul ~0.44 ns/col (>=256 wide), bf16 ~0.505 ns/col
(any width).  f32r for wide phases, bf16 for C (cheap x streaming, narrow
C blocks) and G'.  HBM tensors are host-prearranged to [128, ...] so every
DMA is 128 contiguous descriptors; DMA triggers cost ~700ns serialized on
their queue engine, so output triggers alternate sync/gpsimd.
"""

import numpy as np
import ml_dtypes

import concourse.bass as bass  # noqa: F401  (registers engine namespaces)
import concourse.mybir as mybir
import concourse.tile as tile
from concourse import bacc, bass_utils

F32 = mybir.dt.float32
F32R = mybir.dt.float32r
BF16 = mybir.dt.bfloat16
FP8 = mybir.dt.float8e4
DR = mybir.MatmulPerfMode.DoubleRow

B, N, D, H = 8, 1024, 768, 12
HD = D // H            # 64
SCALE = float(np.sqrt(HD))
DT = D // 128          # 6 chunks of the feature axis
NT = N // 128          # 8 token tiles
NPAIR = H // 2         # 6 head pairs
ECH = 384              # column chunk (fits PSUM bank with headroom)


def _build_program():
    nc = bacc.Bacc(
        trn_type="TRN2", target_bir_lowering=False, debug=False, num_devices=B
    )
    xn_d = nc.dram_tensor("xn", [128, NT, D], FP8, kind="ExternalInput").ap()
    xt_d = nc.dram_tensor("xt", [128, DT, N], F32, kind="ExternalInput").ap()
    wvt_d = nc.dram_tensor("wvt", [128, DT, D], F32, kind="ExternalInput").ap()
    wkt_d = nc.dram_tensor("wkt", [128, DT, D], BF16,
                           kind="ExternalInput").ap()
    wq_d = nc.dram_tensor("wq", [128, DT, D], F32, kind="ExternalInput").ap()
    wfct_d = nc.dram_tensor("wfct", [128, DT, D], F32,
                            kind="ExternalInput").ap()
    idm_d = nc.dram_tensor("idm", [128, 128], BF16, kind="ExternalInput").ap()
    bias_d = nc.dram_tensor("bias", [128, DT], F32, kind="ExternalInput").ap()
    out_d = nc.dram_tensor("outT", [128, 12, 512], BF16,
                           kind="ExternalOutput").ap()

    with tile.TileContext(nc) as tc:
        with tc.tile_pool(name="big", bufs=1) as big, \
             tc.tile_pool(name="tmps", bufs=16) as tmps, \
             tc.tile_pool(name="outsp", bufs=4) as outsp, \
             tc.tile_pool(name="ps", bufs=7, space="PSUM") as ps:

            xn_sb = big.tile([128, NT, D], FP8, name="xn_sb")
            xt_sb = big.tile([128, DT, N], F32R, name="xt_sb")
            wvt_sb = big.tile([128, DT, D], F32R, name="wvt_sb")
            wkt_sb = big.tile([128, DT, D], BF16, name="wkt_sb")
            wq_sb = big.tile([128, DT, D], F32R, name="wq_sb")
            wfct_sb = big.tile([128, DT, D], F32R, name="wfct_sb")
            c_sb = big.tile([128, DT, D], F32R, name="c_sb")
            m1t_sb = big.tile([128, DT, D], BF16, name="m1t_sb")
            g2_sb = big.tile([128, NPAIR, 128], F32R, name="g2_sb")
            gf_sb = big.tile([128, DT, D], F32R, name="gf_sb")
            w_sb = big.tile([128, DT, D], F32R, name="w_sb")
            id_sb = big.tile([128, 128], BF16, name="id_sb")
            bias_sb = big.tile([128, DT], F32, name="bias_sb")
            warm_sb = big.tile([128, 512], BF16, name="warm_sb")

            xtr = xt_d.bitcast(F32R)
            wvtr = wvt_d.bitcast(F32R)
            wqr = wq_d.bitcast(F32R)
            wfctr = wfct_d.bitcast(F32R)

            # ---- DMA triggers: alternate sync/scalar so the ~700ns
            # per-trigger costs overlap; x first (C needs it), small
            # leading chunks so streaming keeps pace with pass 1 ----
            nc.sync.dma_start(xn_sb[:, 0:2, :], xn_d[:, 0:2, :])
            nc.scalar.dma_start(xn_sb[:, 2:5, :], xn_d[:, 2:5, :])
            nc.sync.dma_start(xn_sb[:, 5:8, :], xn_d[:, 5:8, :])
            nc.scalar.dma_start(id_sb[:], idm_d[:])
            nc.sync.dma_start(wvt_sb[:], wvtr[:])
            nc.scalar.dma_start(wkt_sb[:], wkt_d[:])
            nc.sync.dma_start(wfct_sb[:], wfctr[:])
            nc.scalar.dma_start(wq_sb[:], wqr[:])
            nc.sync.dma_start(xt_sb[:], xtr[:])
            nc.scalar.dma_start(bias_sb[:], bias_d[:])

            # ---- PE warmup (p-state ramp) while x streams in ----
            nc.vector.memset(warm_sb[:], 0.0)
            wpt = ps.tile([128, 512], F32, tag="b", name="wpt")
            for i in range(7):
                nc.tensor.matmul(wpt[:], warm_sb[:, 0:128], warm_sb[:],
                                 start=(i == 0), stop=(i == 6))

            # ---- C = x^T x, upper triangle in two passes (bf16).
            # Pass 1 (rows 0-2, cols 384:768) consumes x at roughly its
            # DMA arrival rate; pass 2 covers the rest of the upper
            # triangle starting at the diagonal.  Lower blocks come from
            # PE transposes emitted inside the M1T descent so the chains
            # that need them (small d, late) never wait.
            tmt = {}   # (d1, d2) -> bf16 staging tile for transpose

            def c_pass(specs):
                # specs: list of (d1, cs, ce) accumulator strips
                accs = {}
                for d1, cs, ce in specs:
                    acc = ps.tile([128, ce - cs], F32, tag="b",
                                  name=f"cp{d1}_{cs}")
                    accs[d1] = (acc, cs)
                for t in range(NT // 2):
                    for d1, cs, ce in specs:
                        acc = accs[d1][0]
                        nc.tensor.matmul(
                            acc[:],
                            xn_sb[:, 2 * t:2 * t + 2, d1 * 128:(d1 + 1) * 128],
                            xn_sb[:, 2 * t:2 * t + 2, cs:ce],
                            start=(t == 0), stop=(t == NT // 2 - 1),
                            perf_mode=DR,
                        )
                return accs

            def c_pass_copies(accs, ce_of, copy_order):
                # copy_order: list of ('i', d1) full-strip or ('t', d1, d2)
                # transpose-staging entries, alternating vector/scalar
                for i, ent in enumerate(copy_order):
                    eng_v = (i % 2 == 0)
                    if ent[0] == 'i':
                        d1 = ent[1]
                        acc, cs = accs[d1]
                        dst = c_sb[:, d1, cs:ce_of[d1]]
                        if eng_v:
                            nc.vector.tensor_copy(dst, acc[:])
                        else:
                            nc.scalar.copy(dst, acc[:])
                    else:
                        d1, d2 = ent[1], ent[2]
                        acc, cs = accs[d1]
                        tm = tmps.tile([128, 128], BF16, tag="tm",
                                       name=f"tm{d1}{d2}")
                        src = acc[:, d2 * 128 - cs:(d2 + 1) * 128 - cs]
                        if eng_v:
                            nc.vector.tensor_copy(tm[:], src)
                        else:
                            nc.scalar.copy(tm[:], src)
                        tmt[(d1, d2)] = tm

            def c_transpose(pairs):
                for d1, d2 in pairs:
                    pt = ps.tile([128, 128], BF16, tag="b", name=f"tp{d1}{d2}")
                    nc.tensor.transpose(pt[:], tmt.pop((d1, d2)), id_sb[:])
                    dst = c_sb[:, d2, d1 * 128:(d1 + 1) * 128]
                    if d2 % 2 == 0:
                        nc.scalar.copy(dst, pt[:])
                    else:
                        nc.vector.tensor_copy(dst, pt[:])

            accs_1 = c_pass([(0, ECH, D), (1, ECH, D), (2, ECH, D)])
            c_pass_copies(accs_1, {0: D, 1: D, 2: D},
                          [('i', 0), ('t', 0, 3), ('t', 0, 4), ('t', 0, 5),
                           ('i', 1), ('t', 1, 3), ('t', 1, 4), ('t', 1, 5),
                           ('i', 2), ('t', 2, 3), ('t', 2, 4), ('t', 2, 5)])
            accs_2 = c_pass([(5, 640, D), (4, 512, D), (3, ECH, D),
                             (0, 0, ECH), (1, 128, ECH), (2, 256, ECH)])
            c_pass_copies(accs_2, {5: D, 4: D, 3: D, 0: ECH, 1: ECH, 2: ECH},
                          [('i', 5), ('i', 4), ('t', 4, 5),
                           ('i', 3), ('t', 3, 4), ('t', 3, 5),
                           ('i', 0), ('t', 0, 1), ('t', 0, 2),
                           ('i', 1), ('t', 1, 2), ('i', 2)])

            # pass-1-sourced transposes (staging ready since mid-pass-2);
            # the six pass-2-sourced ones run after M1T d=5, whose 2.1us
            # shadow lets their staging copies land
            c_transpose([(2, 3), (2, 4), (2, 5), (1, 3), (1, 4), (1, 5),
                         (0, 3), (0, 4), (0, 5)])

            # ---- M1T = C @ wv^T (f32r); d descending; per-d k order
            # puts freshly transposed lower blocks at the chain's end ----
            m1_korder = {
                5: [5, 4, 3, 2, 1, 0],
                4: [4, 3, 2, 1, 0, 5],
                3: [3, 2, 1, 0, 4, 5],
            }
            for d in range(DT - 1, -1, -1):
                for jc in range(2):
                    pt = ps.tile([128, ECH], F32, tag="b", name="pt_m1")
                    ks = m1_korder.get(d, list(range(DT - 1, -1, -1)))
                    for i, k in enumerate(ks):
                        nc.tensor.matmul(
                            pt[:],
                            c_sb[:, k, d * 128:(d + 1) * 128],
                            wvt_sb[:, k, jc * ECH:(jc + 1) * ECH],
                            start=(i == 0), stop=(i == DT - 1),
                        )
                    dst = m1t_sb[:, d, jc * ECH:(jc + 1) * ECH]
                    if jc == 0:
                        nc.vector.tensor_copy(dst, pt[:])
                    else:
                        nc.scalar.copy(dst, pt[:])
                if d == DT - 1:
                    c_transpose([(4, 5), (3, 4), (3, 5),
                                 (0, 1), (0, 2), (1, 2)])

            # ---- G' per head pair: diag 64-blocks of M1T_p^T @ wkT_p ----
            for t in range(NPAIR):
                gp = ps.tile([128, 128], F32, tag="b", name="gp")
                for k in range(DT - 1, -1, -1):
                    nc.tensor.matmul(
                        gp[:],
                        m1t_sb[:, k, t * 128:(t + 1) * 128],
                        wkt_sb[:, k, t * 128:(t + 1) * 128],
                        start=(k == DT - 1), stop=(k == 0),
                    )
                nc.vector.tensor_scalar_mul(g2_sb[:, t, :], gp[:], 0.0)
                nc.vector.tensor_copy(g2_sb[0:64, t, 0:64], gp[0:64, 0:64])
                nc.vector.tensor_copy(g2_sb[64:128, t, 64:128],
                                      gp[64:128, 64:128])

            # ---- GF = blockdiag(G') @ wfc^T (f32r) ----
            for jc in range(2):
                for t in range(NPAIR):
                    pt = ps.tile([128, ECH], F32, tag="b", name="pt_gf")
                    nc.tensor.matmul(
                        pt[:],
                        g2_sb[:, t, :],
                        wfct_sb[:, t, jc * ECH:(jc + 1) * ECH],
                        start=True, stop=True,
                    )
                    dst = gf_sb[:, t, jc * ECH:(jc + 1) * ECH]
                    if t % 2 == 0:
                        nc.vector.tensor_copy(dst, pt[:])
                    else:
                        nc.scalar.copy(dst, pt[:])

            # ---- W = (s*wq)^T @ GF (f32r) ----
            for jc in range(2):
                for d in range(DT):
                    pt = ps.tile([128, ECH], F32, tag="b", name="pt_w")
                    for k in range(DT):
                        nc.tensor.matmul(
                            pt[:],
                            wq_sb[:, k, d * 128:(d + 1) * 128],
                            gf_sb[:, k, jc * ECH:(jc + 1) * ECH],
                            start=(k == 0), stop=(k == DT - 1),
                        )
                    dst = w_sb[:, d, jc * ECH:(jc + 1) * ECH]
                    if d % 2 == 0:
                        nc.vector.tensor_copy(dst, pt[:])
                    else:
                        nc.scalar.copy(dst, pt[:])

            # ---- outT[j, n] = sum_d W[d, j] x^T[d, n] + bias ----
            for jt in range(DT):
                for ic in range(2):
                    last = (jt == DT - 1 and ic == 1)
                    if not last:
                        pt = ps.tile([128, 512], F32, tag="b", name="pt_o")
                        for k in range(DT):
                            nc.tensor.matmul(
                                pt[:],
                                w_sb[:, k, jt * 128:(jt + 1) * 128],
                                xt_sb[:, k, ic * 512:(ic + 1) * 512],
                                start=(k == 0), stop=(k == DT - 1),
                            )
                        ot = outsp.tile([128, 512], BF16, tag="ot",
                                        name="ot")
                        # add + trigger both on scalar: no cross-engine hop
                        nc.scalar.add(ot[:], pt[:], bias_sb[:, jt:jt + 1])
                        nc.scalar.dma_start(out_d[:, jt * 2 + ic, :], ot[:])
                    else:
                        # final piece in two halves so the first DMA
                        # overlaps the second half's matmuls
                        for h in range(2):
                            off = ic * 512 + h * 256
                            pt = ps.tile([128, 256], F32, tag="b",
                                         name="pt_ol")
                            for k in range(DT):
                                nc.tensor.matmul(
                                    pt[:],
                                    w_sb[:, k, jt * 128:(jt + 1) * 128],
                                    xt_sb[:, k, off:off + 256],
                                    start=(k == 0), stop=(k == DT - 1),
                                )
                            ot = outsp.tile([128, 256], BF16, tag="otl",
                                            bufs=2, name="otl")
                            if h == 0:
                                nc.scalar.add(ot[:], pt[:],
                                              bias_sb[:, jt:jt + 1])
                                nc.scalar.dma_start(
                                    out_d[:, jt * 2 + ic, 0:256], ot[:])
                            else:
                                nc.vector.tensor_scalar_add(
                                    ot[:], pt[:], bias_sb[:, jt:jt + 1])
                                nc.sync.dma_start(
                                    out_d[:, jt * 2 + ic, 256:512], ot[:])

    nc.compile()
    return nc


_NC_CACHE = None
LAST_EXEC_NS = None
LAST_RES = None


def _arr128(a):
    """[D0*128, M] row-major -> [128, D0, M] partition-major, contiguous."""
    d0 = a.shape[0] // 128
    return np.ascontiguousarray(
        a.reshape(d0, 128, a.shape[1]).transpose(1, 0, 2))


def kernel(x, w_qkv, w_fc, b_fc, _trace=False):
    global _NC_CACHE, LAST_EXEC_NS, LAST_RES
    x = np.asarray(x, dtype=np.float32)
    w_qkv = np.asarray(w_qkv, dtype=np.float32)
    w_fc = np.asarray(w_fc, dtype=np.float32)
    b_fc = np.asarray(b_fc, dtype=np.float32)

    if _NC_CACHE is None:
        _NC_CACHE = _build_program()
    nc = _NC_CACHE

    bf = ml_dtypes.bfloat16
    wq = _arr128(SCALE * w_qkv[:D])                       # [128, 6, 768] f32
    wkt = _arr128(np.ascontiguousarray(w_qkv[D:2 * D].T).astype(bf))
    wvt = _arr128(np.ascontiguousarray(w_qkv[2 * D:].T))
    wfct = _arr128(np.ascontiguousarray(w_fc.T))
    bias = np.ascontiguousarray(b_fc.reshape(DT, 128).T)  # [128, 6] f32
    idm = np.eye(128, dtype=bf)

    in_maps = []
    for b in range(B):
        in_maps.append({
            "xn": _arr128(x[b].astype(ml_dtypes.float8_e4m3fn)),
            "xt": _arr128(np.ascontiguousarray(x[b].T)),  # [128, 6, 1024] f32
            "wvt": wvt, "wkt": wkt, "wq": wq, "wfct": wfct,
            "idm": idm, "bias": bias,
        })

    res = bass_utils.run_bass_kernel_spmd(
        nc, in_maps, core_ids=list(range(B)), trace=_trace
    )
    LAST_EXEC_NS = res.exec_time_ns
    LAST_RES = res
    outs = []
    for b in range(B):
        a = res.results[b]["outT"]                        # [128, 12, 512] bf16
        a = a.reshape(128, DT, 2, 512).transpose(1, 0, 2, 3).reshape(D, N)
        outs.append(a.T.astype(np.float32))
    return np.ascontiguousarray(np.stack(outs))
